# revision 15
# baseline (speedup 1.0000x reference)
"""Trainium2 Bass kernel for the DifferentiableCBFLayer batched dual-FISTA QP.

v2 strategy (pure data parallel, 8 cores x 4096 samples):
  Each core's 4096 samples live as [128 partitions, 32 free] f32 planes.
  The free dim is split into two independent 16-column groups; each group
  runs the full pipeline on its own tiles so the Tile scheduler can
  phase-offset them across engines (DVE + GPSIMD/Pool working together,
  no per-iteration ACT involvement).

  Per-group FISTA iteration (22 active dual rows -- the 3 slack rows are
  provably ~0 (<=2.3e-3 over the whole trajectory) and are dropped from
  the loop; they are kept in the power phase where they shape L):
    fwd:  prod = W o bc(y[0:18]); S01 via log-tree of TT adds (no strided
          tensor_reduce -- measured 1.9 ns/elem vs 1.04 for contiguous TT);
          R2/R3 trees; bx/R01; X = SS o R + Qadd
    bwd:  Bprod = W o bc(X01); pair add; group shifts
    arg:  arg[0:18] = T + y; box rows via 2 STTs (+-K0*X01 + y)
    upd:  lam' = max(arg,0) - btil (STT);
          y' = (1+b)lam' - b lam  (ts mult + STT)
  Heavy TT ops (F1/B1 products) run on the Pool engine; everything else
  on DVE. The power phase normalizes by ||T||^2 (reciprocal, no ACT sqrt)
  which only changes the iterate's scale, not its direction.
"""
import os
from contextlib import ExitStack

import numpy as np

import concourse.bass as bass
import concourse.tile as tile
from concourse import mybir
from concourse.bass_utils import run_bass_kernel_spmd

f32 = mybir.dt.float32
AX = mybir.AxisListType
OP = mybir.AluOpType
AF = mybir.ActivationFunctionType

P = 128
F = 32                 # samples per partition (full width)
NG = 2                 # groups
FH = F // NG           # samples per partition per group
BPC = P * F            # samples per core
NCORES = 8
B_FULL = BPC * NCORES  # 32768

N_POWER = 30
N_FISTA = 300
USE_POOL = True

MAX_OBS = 10
MAX_NEI = 7
BIG = 1000.0
PINV = np.array([0.5, 0.5, 1.0 / 200.0, 1.0 / 200.0, 1.0 / 200.0], np.float64)
K0 = float(np.float32(np.sqrt(PINV[0])))      # sqrt(1/2)
PINV2 = float(np.float32(PINV[2]))
SQ2 = float(np.float32(np.sqrt(2.0)))

RAW_SPECS = [
    ("u_nominal", (BPC, 2)),
    ("v_current", (BPC, 1)),
    ("p_obs", (BPC, MAX_OBS, 2)),
    ("obs_mask", (BPC, MAX_OBS)),
    ("p_agents", (BPC, MAX_NEI, 2)),
    ("v_agents_local", (BPC, MAX_NEI, 2)),
    ("agents_mask", (BPC, MAX_NEI)),
    ("p_c_agent", (BPC, 1, 2)),
    ("v_c_agent", (BPC, 1, 2)),
    ("closest_mask", (BPC, 1)),
]
NFEAT = 73
F_U, F_V, F_OBS, F_OM, F_AG, F_VA, F_AM, F_PC, F_VC, F_CM = 0, 2, 3, 23, 33, 47, 61, 68, 70, 72

# FISTA-loop row layout (22 rows): 0-9 obs, 10-16 nei, 17 cn, 18-21 box
# (box order: -a, +a, -w, +w as in the reference).
NL = 22
# power-phase layout keeps all 25 rows: 0-17 geom, 18-20 slack, 21-24 box.


def _betas(n):
    one, half, four = np.float32(1.0), np.float32(0.5), np.float32(4.0)
    tk = np.float32(1.0)
    out = []
    for _ in range(n):
        tk1 = half * (one + np.sqrt(one + four * tk * tk, dtype=np.float32))
        beta = np.float32((tk - one) / tk1)
        out.append(float(beta))
        tk = tk1
    return out


class EmitG:
    """One sample-group (FH free columns) with its own tiles."""

    def __init__(self, ctx, tc, natt, gi):
        self.tc = tc
        self.nc = tc.nc
        self.gi = gi
        self.natt = natt
        pool = ctx.enter_context(tc.tile_pool(name=f"st{gi}", bufs=1))
        T = lambda n, tag: pool.tile([P, n * FH], f32, name=f"{tag}{gi}", tag=f"{tag}{gi}")
        self.W = T(36, "W")          # Abar cols 0,1, (row, col) pair-major
        self.b = T(25, "b")          # unscaled h (power layout)
        self.btil = T(NL, "btil")    # sqrt(s) * b, loop layout
        self.lamA = T(NL, "lamA")
        self.lamB = T(NL, "lamB")
        self.yh = T(25, "yh")        # loop y (22 used); power z (25)
        self.Tbuf = T(25, "Tbuf")    # backward output
        self.arg = T(NL, "arg")
        self.pscr = T(25, "pscr")    # precompute/power scratch
        self.tsc = T(18, "tsc")      # tree scratch
        # Separate fixed product buffers for F1/B1. Pool-sited ops alternate
        # engines by iteration parity, so every buffer hazard is
        # cross-engine and each instruction needs <=1 sync wait (walrus
        # limit; the Pool engine is out-of-order, so same-engine Pool deps
        # would otherwise synthesize a second wait).
        self.prodF = T(36, "prodF")
        self.prodB = T(36, "prodB")
        self.X = T(5, "X")
        self.R = T(5, "R")
        self.SS = T(5, "SS")
        self.SSp = T(5, "SSp")
        self.Qadd = T(5, "Qadd")
        self.cs2 = T(2, "cs2")
        self.bx = T(2, "bx")
        self.u2 = T(2, "u2")
        self.sc1 = T(1, "sc1")
        self.ns = T(1, "ns")
        self.den = T(1, "den")
        self.rn = T(1, "rn")
        self.rs = T(1, "rs")
        self.sstar = T(1, "sstar")
        self.v2 = T(1, "v2")
        self.opack = T(2, "opack")
        self.fence_pool = ctx.enter_context(tc.tile_pool(name=f"fence{gi}", bufs=2))

    # ---------------- helpers ----------------
    def _pl(self, t, i, n=1):
        return t[:, i * FH:(i + n) * FH]

    def _pv(self, t, i, n):
        """[P, n, FH] view of planes [i, i+n)."""
        return t[:, i * FH:(i + n) * FH].rearrange("p (m f) -> p m f", m=n)

    def _bc(self, plane_ap, n):
        return plane_ap.unsqueeze(1).broadcast_to([P, n, FH])

    def _act_fence(self, act_insts):
        ft = self.fence_pool.tile([P, 1], f32, name=f"fence{self.gi}", tag=f"fence{self.gi}")
        ins = self.nc.vector.memset(ft[:], 0.0)
        for ai in act_insts:
            tile.add_dep_helper(ins.ins, ai.ins, sync=True, reason="act fence")
        return ins

    # ---------------- fwd/bwd machinery ----------------
    def tree10(self, z, zoff, out_pl):
        """out = sum of 10 z planes [zoff, zoff+10) via 4 TT adds (DVE)."""
        TT = self.nc.vector.tensor_tensor
        t = self.tsc
        TT(out=self._pv(t, 0, 5), in0=self._pv(z, zoff, 5),
           in1=self._pv(z, zoff + 5, 5), op=OP.add)
        TT(out=self._pv(t, 0, 2), in0=self._pv(t, 0, 2),
           in1=self._pv(t, 2, 2), op=OP.add)
        TT(out=self._pl(t, 0), in0=self._pl(t, 0), in1=self._pl(t, 1), op=OP.add)
        TT(out=out_pl, in0=self._pl(t, 0), in1=self._pl(t, 4), op=OP.add)

    def tree7(self, z, zoff, out_pl):
        """out = sum of 7 z planes [zoff, zoff+7) via 4 TT adds (DVE).
        Uses tsc planes 5-8 (tree10's result planes 0,4 stay live)."""
        TT = self.nc.vector.tensor_tensor
        t = self.tsc
        TT(out=self._pv(t, 5, 3), in0=self._pv(z, zoff, 3),
           in1=self._pv(z, zoff + 3, 3), op=OP.add)
        TT(out=self._pl(t, 5), in0=self._pl(t, 5), in1=self._pl(t, 6), op=OP.add)
        TT(out=self._pl(t, 6), in0=self._pl(t, 7), in1=self._pl(z, zoff + 6), op=OP.add)
        TT(out=out_pl, in0=self._pl(t, 5), in1=self._pl(t, 6), op=OP.add)

    def forward(self, z, SSbuf, Qadd, X, slack, peng=None):
        """X = SS o R(z) (+Qadd). slack=True: 25-row power layout."""
        nc = self.nc
        TT = nc.vector.tensor_tensor
        PE = (peng or nc.vector).tensor_tensor
        prod = self.prodF
        W4 = self.W[:].rearrange("p (r c f) -> p r c f", r=18, c=2)
        P4 = prod[:].rearrange("p (r c f) -> p r c f", r=18, c=2)
        # products for the 18 geometric rows (optionally on Pool)
        PE(out=P4, in0=W4,
           in1=self._pv(z, 0, 18).unsqueeze(2).broadcast_to([P, 18, 2, FH]),
           op=OP.mult)
        # S01 = column sums over 18 rows via tree on (r) axis, c pairs together.
        # prod pair-major planes: row r occupies planes 2r, 2r+1.
        pv = lambda i, n: prod[:, i * FH:(i + n) * FH].rearrange("p (m f) -> p m f", m=n)
        TT(out=self._pv(self.tsc, 0, 18), in0=pv(0, 18),
           in1=pv(18, 18), op=OP.add)                      # 9 rows x 2 planes
        TT(out=self._pv(self.tsc, 0, 8), in0=self._pv(self.tsc, 0, 8),
           in1=self._pv(self.tsc, 8, 8), op=OP.add)        # 4 rows
        TT(out=self._pv(self.tsc, 0, 4), in0=self._pv(self.tsc, 0, 4),
           in1=self._pv(self.tsc, 4, 4), op=OP.add)        # 2 rows
        TT(out=self._pv(self.tsc, 0, 2), in0=self._pv(self.tsc, 0, 2),
           in1=self._pv(self.tsc, 2, 2), op=OP.add)        # 1 row
        TT(out=self.cs2[:].rearrange("p (c f) -> p c f", c=2),
           in0=self._pv(self.tsc, 0, 2), in1=self._pv(self.tsc, 16, 2), op=OP.add)
        # box diffs: loop rows 18-21 (-a,+a,-w,+w) / power rows 21-24
        bo = 18 if not slack else 21
        zbox = z[:, bo * FH:(bo + 4) * FH].rearrange("p (c g f) -> p c g f", c=2, g=2)
        TT(out=self.bx[:].rearrange("p (c f) -> p c f", c=2),
           in0=zbox[:, :, 1, :], in1=zbox[:, :, 0, :], op=OP.subtract)
        nc.vector.scalar_tensor_tensor(
            out=self._pl(self.R, 0, 2), in0=self.bx[:], scalar=K0, in1=self.cs2[:],
            op0=OP.mult, op1=OP.add)
        # R2 = sum(z[0:10]) (+z18 if slack); R3 = sum(z[10:17]) (+z19)
        self.tree10(z, 0, self._pl(self.R, 2))
        self.tree7(z, 10, self._pl(self.R, 3))
        if slack:
            TT(out=self._pl(self.R, 2), in0=self._pl(self.R, 2), in1=self._pl(z, 18), op=OP.add)
            TT(out=self._pl(self.R, 3), in0=self._pl(self.R, 3), in1=self._pl(z, 19), op=OP.add)
            TT(out=self._pl(self.R, 4), in0=self._pl(z, 17), in1=self._pl(z, 20), op=OP.add)
            nR = 5
        else:
            nR = 4
        # X = SS * R (+ Qadd); loop: X4 = SS4 * z17 handled separately
        TT(out=self._pl(X, 0, nR), in0=self._pl(SSbuf, 0, nR), in1=self._pl(self.R, 0, nR), op=OP.mult)
        if not slack:
            TT(out=self._pl(X, 4), in0=self._pl(SSbuf, 4), in1=self._pl(z, 17), op=OP.mult)
        if Qadd is not None:
            TT(out=X[:], in0=X[:], in1=Qadd[:], op=OP.add)

    def backward_geom(self, X, peng=None):
        """Tbuf[0:18] = W o X01 pair-sum minus group shifts."""
        nc = self.nc
        TT = nc.vector.tensor_tensor
        PE = (peng or nc.vector).tensor_tensor
        Tb = self.Tbuf
        prod = self.prodB
        W4 = self.W[:].rearrange("p (r c f) -> p r c f", r=18, c=2)
        P4 = prod[:].rearrange("p (r c f) -> p r c f", r=18, c=2)
        x2 = X[:, 0:2 * FH].rearrange("p (c f) -> p c f", c=2)
        PE(out=P4, in0=W4,
           in1=x2.unsqueeze(1).broadcast_to([P, 18, 2, FH]), op=OP.mult)
        TT(out=self._pv(Tb, 0, 18),
           in0=P4[:, :, 0, :], in1=P4[:, :, 1, :], op=OP.add)
        TT(out=self._pv(Tb, 0, 10), in0=self._pv(Tb, 0, 10),
           in1=self._bc(self._pl(X, 2), 10), op=OP.subtract)
        TT(out=self._pv(Tb, 10, 7), in0=self._pv(Tb, 10, 7),
           in1=self._bc(self._pl(X, 3), 7), op=OP.subtract)
        TT(out=self._pl(Tb, 17), in0=self._pl(Tb, 17), in1=self._pl(X, 4), op=OP.subtract)

    def backward_tail_power(self, X):
        """Power-phase slack+box rows of Tbuf (25-row layout) on DVE."""
        nc = self.nc
        ts = nc.vector.tensor_scalar
        ts(out=self._pl(self.Tbuf, 18, 3), in0=self._pl(X, 2, 3), scalar1=-1.0,
           scalar2=None, op0=OP.mult)
        tb = self.Tbuf[:, 21 * FH:25 * FH].rearrange("p (c g f) -> p c g f", c=2, g=2)
        x2 = X[:, 0:2 * FH].rearrange("p (c f) -> p c f", c=2)
        ts(out=tb[:, :, 0, :], in0=x2, scalar1=-K0, scalar2=None, op0=OP.mult)
        ts(out=tb[:, :, 1, :], in0=x2, scalar1=K0, scalar2=None, op0=OP.mult)

    # ---------------- precompute (per group) ----------------
    def precompute(self):
        nc = self.nc
        gsl = slice(self.gi * FH, (self.gi + 1) * FH)
        Wv = self.W[:].rearrange("p (m f) -> p m f", m=36)
        bv = self.b[:].rearrange("p (m f) -> p m f", m=25)
        STT = nc.vector.scalar_tensor_tensor
        TT = nc.vector.tensor_tensor
        TS = nc.vector.tensor_scalar

        pk = self.natt[:].rearrange("p (f a) -> p a f", a=NFEAT)[:, :, gsl]
        self.pk = pk
        obs = pk[:, F_OBS:F_OBS + 20, :].rearrange("p (o c) f -> p o c f", c=2)
        lx, ly = obs[:, :, 0, :], obs[:, :, 1, :]
        om = pk[:, F_OM:F_OM + MAX_OBS, :]
        ag = pk[:, F_AG:F_AG + 14, :].rearrange("p (o c) f -> p o c f", c=2)
        ax, ay = ag[:, :, 0, :], ag[:, :, 1, :]
        va = pk[:, F_VA:F_VA + 14, :].rearrange("p (o c) f -> p o c f", c=2)
        vjx, vjy = va[:, :, 0, :], va[:, :, 1, :]
        am = pk[:, F_AM:F_AM + MAX_NEI, :]
        cx, cy = pk[:, F_PC, :], pk[:, F_PC + 1, :]
        cvx, cvy = pk[:, F_VC, :], pk[:, F_VC + 1, :]
        cm = pk[:, F_CM, :]
        v = pk[:, F_V, :]

        STT(out=self.v2[:], in0=v, scalar=2.0, in1=v, op0=OP.mult, op1=OP.mult)

        sA = self._pv(self.pscr, 0, 10)
        sB = self._pv(self.pscr, 10, 10)

        # ---- obs rows (0-9; W pairs 0..19) ----
        W0 = Wv[:, 0:20].rearrange("p (o c) f -> p o c f", c=2)[:, :, 0, :]
        W1 = Wv[:, 0:20].rearrange("p (o c) f -> p o c f", c=2)[:, :, 1, :]
        STT(out=W0, in0=lx, scalar=2.0 * K0, in1=om, op0=OP.mult, op1=OP.mult)
        STT(out=sA, in0=ly, scalar=2.0 * K0, in1=self._bc(v, 10), op0=OP.mult, op1=OP.mult)
        TT(out=W1, in0=sA, in1=om, op=OP.mult)
        STT(out=sA, in0=lx, scalar=-6.0, in1=self._bc(v, 10), op0=OP.mult, op1=OP.mult)
        STT(out=sB, in0=lx, scalar=2.0, in1=lx, op0=OP.mult, op1=OP.mult)
        TT(out=sA, in0=sA, in1=sB, op=OP.add)
        STT(out=sB, in0=ly, scalar=2.0, in1=ly, op0=OP.mult, op1=OP.mult)
        TT(out=sA, in0=sA, in1=sB, op=OP.add)
        TT(out=sA, in0=sA, in1=self._bc(self.v2[:], 10), op=OP.add)
        TS(out=sA, in0=sA, scalar1=-0.5, scalar2=None, op0=OP.add)
        STT(out=sA, in0=sA, scalar=-BIG, in1=om, op0=OP.add, op1=OP.mult)
        TS(out=bv[:, 0:10], in0=sA, scalar1=BIG, scalar2=None, op0=OP.add)

        # ---- nei rows (10-16; W pairs 20..33) ----
        sA7 = self._pv(self.pscr, 0, 7)
        sB7 = self._pv(self.pscr, 7, 7)
        W0 = Wv[:, 20:34].rearrange("p (o c) f -> p o c f", c=2)[:, :, 0, :]
        W1 = Wv[:, 20:34].rearrange("p (o c) f -> p o c f", c=2)[:, :, 1, :]
        STT(out=W0, in0=ax, scalar=2.0 * K0, in1=am, op0=OP.mult, op1=OP.mult)
        STT(out=sA7, in0=ay, scalar=2.0 * K0, in1=self._bc(v, 7), op0=OP.mult, op1=OP.mult)
        STT(out=sB7, in0=ay, scalar=-2.0 * K0, in1=vjx, op0=OP.mult, op1=OP.mult)
        TT(out=sA7, in0=sA7, in1=sB7, op=OP.add)
        STT(out=sB7, in0=ax, scalar=2.0 * K0, in1=vjy, op0=OP.mult, op1=OP.mult)
        TT(out=sA7, in0=sA7, in1=sB7, op=OP.add)
        TT(out=W1, in0=sA7, in1=am, op=OP.mult)
        STT(out=sA7, in0=vjx, scalar=-4.0, in1=self._bc(v, 7), op0=OP.mult, op1=OP.mult)
        STT(out=sB7, in0=vjx, scalar=2.0, in1=vjx, op0=OP.mult, op1=OP.mult)
        TT(out=sA7, in0=sA7, in1=sB7, op=OP.add)
        STT(out=sB7, in0=vjy, scalar=2.0, in1=vjy, op0=OP.mult, op1=OP.mult)
        TT(out=sA7, in0=sA7, in1=sB7, op=OP.add)
        STT(out=sB7, in0=ax, scalar=-6.0, in1=self._bc(v, 7), op0=OP.mult, op1=OP.mult)
        TT(out=sA7, in0=sA7, in1=sB7, op=OP.add)
        STT(out=sB7, in0=ax, scalar=6.0, in1=vjx, op0=OP.mult, op1=OP.mult)
        TT(out=sA7, in0=sA7, in1=sB7, op=OP.add)
        STT(out=sB7, in0=ay, scalar=6.0, in1=vjy, op0=OP.mult, op1=OP.mult)
        TT(out=sA7, in0=sA7, in1=sB7, op=OP.add)
        STT(out=sB7, in0=ax, scalar=2.0, in1=ax, op0=OP.mult, op1=OP.mult)
        TT(out=sA7, in0=sA7, in1=sB7, op=OP.add)
        STT(out=sB7, in0=ay, scalar=2.0, in1=ay, op0=OP.mult, op1=OP.mult)
        TT(out=sA7, in0=sA7, in1=sB7, op=OP.add)
        TT(out=sA7, in0=sA7, in1=self._bc(self.v2[:], 7), op=OP.add)
        TS(out=sA7, in0=sA7, scalar1=-1.28, scalar2=None, op0=OP.add)
        STT(out=sA7, in0=sA7, scalar=-BIG, in1=am, op0=OP.add, op1=OP.mult)
        TS(out=bv[:, 10:17], in0=sA7, scalar1=BIG, scalar2=None, op0=OP.add)

        # ---- cn row (17; W pair 34,35) ----
        s1 = self._pl(self.pscr, 0)
        s2 = self._pl(self.pscr, 1)
        STT(out=Wv[:, 34], in0=cx, scalar=-2.0 * K0, in1=cm, op0=OP.mult, op1=OP.mult)
        STT(out=s1, in0=cy, scalar=-2.0 * K0, in1=v, op0=OP.mult, op1=OP.mult)
        STT(out=s2, in0=cy, scalar=2.0 * K0, in1=cvx, op0=OP.mult, op1=OP.mult)
        TT(out=s1, in0=s1, in1=s2, op=OP.add)
        STT(out=s2, in0=cx, scalar=-2.0 * K0, in1=cvy, op0=OP.mult, op1=OP.mult)
        TT(out=s1, in0=s1, in1=s2, op=OP.add)
        TT(out=Wv[:, 35], in0=s1, in1=cm, op=OP.mult)
        STT(out=s1, in0=cvx, scalar=4.0, in1=v, op0=OP.mult, op1=OP.mult)
        STT(out=s2, in0=cvx, scalar=-2.0, in1=cvx, op0=OP.mult, op1=OP.mult)
        TT(out=s1, in0=s1, in1=s2, op=OP.add)
        STT(out=s2, in0=cvy, scalar=-2.0, in1=cvy, op0=OP.mult, op1=OP.mult)
        TT(out=s1, in0=s1, in1=s2, op=OP.add)
        STT(out=s2, in0=cx, scalar=6.0, in1=v, op0=OP.mult, op1=OP.mult)
        TT(out=s1, in0=s1, in1=s2, op=OP.add)
        STT(out=s2, in0=cx, scalar=-6.0, in1=cvx, op0=OP.mult, op1=OP.mult)
        TT(out=s1, in0=s1, in1=s2, op=OP.add)
        STT(out=s2, in0=cy, scalar=-6.0, in1=cvy, op0=OP.mult, op1=OP.mult)
        TT(out=s1, in0=s1, in1=s2, op=OP.add)
        STT(out=s2, in0=cx, scalar=-2.0, in1=cx, op0=OP.mult, op1=OP.mult)
        TT(out=s1, in0=s1, in1=s2, op=OP.add)
        STT(out=s2, in0=cy, scalar=-2.0, in1=cy, op0=OP.mult, op1=OP.mult)
        TT(out=s1, in0=s1, in1=s2, op=OP.add)
        TT(out=s1, in0=s1, in1=self.v2[:], op=OP.subtract)
        TS(out=s1, in0=s1, scalar1=50.0, scalar2=None, op0=OP.add)
        STT(out=s1, in0=s1, scalar=-BIG, in1=cm, op0=OP.add, op1=OP.mult)
        TS(out=self._pl(self.b, 17), in0=s1, scalar1=BIG, scalar2=None, op0=OP.add)

        # ---- slack/box b rows (power layout), power scale planes ----
        nc.vector.memset(self._pl(self.b, 18, 3), 0.0)
        nc.vector.memset(self._pl(self.b, 21, 2), 2.0)
        nc.vector.memset(self._pl(self.b, 23, 2), 1.0)
        nc.vector.memset(self._pl(self.SSp, 0, 2), 1.0)
        nc.vector.memset(self._pl(self.SSp, 2, 3), -PINV2)

    # ---------------- power phase ----------------
    def power_phase(self, n_power):
        nc = self.nc
        TT = nc.vector.tensor_tensor
        nc.vector.memset(self.yh[:], 1.0)
        sq = self.pscr[:, 0:25 * FH]
        for it in range(n_power):
            par = (it + self.gi) % 2
            f1eng = nc.gpsimd if (USE_POOL and par == 0) else None
            b1eng = nc.gpsimd if (USE_POOL and par == 1) else None
            self.forward(self.yh, self.SSp, None, self.X, slack=True, peng=f1eng)
            self.backward_geom(self.X, peng=b1eng)
            self.backward_tail_power(self.X)
            # z' = T / ||T||^2  (direction-preserving, no sqrt needed)
            TT(out=sq, in0=self.Tbuf[:], in1=self.Tbuf[:], op=OP.mult)
            nc.vector.tensor_reduce(
                out=self.ns[:], in_=sq.rearrange("p (m f) -> p f m", m=25),
                axis=AX.X, op=OP.add)
            nc.vector.reciprocal(out=self.rn[:], in_=self.ns[:])
            TT(out=self._pv(self.yh, 0, 25), in0=self._pv(self.Tbuf, 0, 25),
               in1=self._bc(self.rn[:], 25), op=OP.mult)
        # Rayleigh: L = (z.Mz)/(z.z); ns := L + 1e-6; rs = rsqrt(ns)
        self.forward(self.yh, self.SSp, None, self.X, slack=True)
        self.backward_geom(self.X)
        self.backward_tail_power(self.X)
        TT(out=sq, in0=self.yh[:], in1=self.Tbuf[:], op=OP.mult)
        nc.vector.tensor_reduce(
            out=self.ns[:], in_=sq.rearrange("p (m f) -> p f m", m=25),
            axis=AX.X, op=OP.add)
        TT(out=sq, in0=self.yh[:], in1=self.yh[:], op=OP.mult)
        nc.vector.tensor_reduce(
            out=self.den[:], in_=sq.rearrange("p (m f) -> p f m", m=25),
            axis=AX.X, op=OP.add)
        nc.vector.reciprocal(out=self.rn[:], in_=self.den[:])
        TT(out=self.ns[:], in0=self.ns[:], in1=self.rn[:], op=OP.mult)
        nc.vector.tensor_scalar(out=self.ns[:], in0=self.ns[:],
                                scalar1=1e-6, scalar2=None, op0=OP.add)
        self.emit_rsqrt(self.rs[:], self.ns[:], newton=2)
        TT(out=self.sstar[:], in0=self.rs[:], in1=self.rs[:], op=OP.mult)

    def emit_rsqrt(self, dst, src, newton=0):
        nc = self.nc
        nc.vector.reciprocal(out=self.sc1[:], in_=src)
        a = nc.scalar.activation(dst, self.sc1[:], AF.Sqrt)
        self.last_act = a
        self._act_fence([a])
        for _ in range(newton):
            nc.vector.tensor_tensor(out=self.sc1[:], in0=dst, in1=dst, op=OP.mult)
            nc.vector.tensor_tensor(out=self.sc1[:], in0=src, in1=self.sc1[:], op=OP.mult)
            nc.vector.tensor_scalar(out=self.sc1[:], in0=self.sc1[:],
                                    scalar1=-0.5, scalar2=1.5, op0=OP.mult, op1=OP.add)
            nc.vector.tensor_tensor(out=dst, in0=dst, in1=self.sc1[:], op=OP.mult)

    # ---------------- FISTA setup ----------------
    def fista_setup(self):
        """btil (22-row layout), SS, Qadd, lam0=y0=-btil."""
        nc = self.nc
        TT = nc.vector.tensor_tensor
        ts = nc.vector.tensor_scalar
        # btil loop layout: rows 0-17 from b[0:18], rows 18-21 from b[21:25]
        TT(out=self._pv(self.btil, 0, 18), in0=self._pv(self.b, 0, 18),
           in1=self._bc(self.rs[:], 18), op=OP.mult)
        TT(out=self._pv(self.btil, 18, 4), in0=self._pv(self.b, 21, 4),
           in1=self._bc(self.rs[:], 4), op=OP.mult)
        # SS = [-s*, -s*, PINV2*s* x3]
        ts(out=self._pv(self.SS, 0, 2), in0=self._bc(self.sstar[:], 2),
           scalar1=-1.0, scalar2=None, op0=OP.mult)
        ts(out=self._pv(self.SS, 2, 3), in0=self._bc(self.sstar[:], 3),
           scalar1=PINV2, scalar2=None, op0=OP.mult)
        # Qadd = SS*FWD25(btil-as-25rows) + rs*q~ ; build a 25-row view of btil
        # (slack rows zero): reuse Tbuf as scratch z.
        nc.vector.memset(self.Tbuf[:], 0.0)
        ts(out=self._pv(self.Tbuf, 0, 18), in0=self._pv(self.btil, 0, 18),
           scalar1=1.0, scalar2=None, op0=OP.mult)
        ts(out=self._pv(self.Tbuf, 21, 4), in0=self._pv(self.btil, 18, 4),
           scalar1=1.0, scalar2=None, op0=OP.mult)
        self.forward(self.Tbuf, self.SS, None, self.Qadd, slack=True)
        uap = self.pk[:, F_U:F_U + 2, :]
        nc.vector.scalar_tensor_tensor(
            out=self.u2[:].rearrange("p (c f) -> p c f", c=2),
            in0=uap, scalar=SQ2, in1=self._bc(self.rs[:], 2), op0=OP.mult, op1=OP.mult)
        TT(out=self._pl(self.Qadd, 0, 2), in0=self._pl(self.Qadd, 0, 2), in1=self.u2[:],
           op=OP.add)
        # lam = y = -btil (22 rows)
        ts(out=self.lamA[:, 0:NL * FH], in0=self.btil[:], scalar1=-1.0, scalar2=None, op0=OP.mult)
        ts(out=self.yh[:, 0:NL * FH], in0=self.btil[:], scalar1=-1.0, scalar2=None, op0=OP.mult)

    # ---------------- FISTA loop ----------------
    def fista(self, n_fista):
        nc = self.nc
        TT = nc.vector.tensor_tensor
        STT = nc.vector.scalar_tensor_tensor
        ts = nc.vector.tensor_scalar
        betas = _betas(n_fista)
        lams = [self.lamA, self.lamB]
        yl = self.yh[:, 0:NL * FH]
        for it in range(n_fista):
            lam_prev = lams[it % 2]
            lam_new = lams[(it + 1) % 2]
            beta = float(np.float32(betas[it]))
            par = (it + self.gi) % 2
            f1eng = nc.gpsimd if (USE_POOL and par == 0) else None
            b1eng = nc.gpsimd if (USE_POOL and par == 1) else None
            self.forward(self.yh, self.SS, self.Qadd, self.X, slack=False,
                         peng=f1eng)
            self.backward_geom(self.X, peng=b1eng)
            arg = self.arg
            # arg rows 0-17: T + y  (alternates DVE/Pool by parity)
            u1eng = nc.gpsimd if (USE_POOL and par == 1) else nc.vector
            ins = u1eng.tensor_tensor(
                out=arg[:, 0:18 * FH].rearrange("p (m f) -> p m f", m=18),
                in0=self._pv(self.Tbuf, 0, 18),
                in1=self._pv(self.yh, 0, 18), op=OP.add)
            if USE_POOL and par == 1:
                self.last_pool = ins
            # arg box rows: -+K0*X01 + y  (planes 18-21 as [c=a/w, g=-/+])
            x2 = self.X[:, 0:2 * FH].rearrange("p (c f) -> p c f", c=2)
            ab = arg[:, 18 * FH:22 * FH].rearrange("p (c g f) -> p c g f", c=2, g=2)
            yb = self.yh[:, 18 * FH:22 * FH].rearrange("p (c g f) -> p c g f", c=2, g=2)
            STT(out=ab[:, :, 0, :], in0=x2, scalar=-K0, in1=yb[:, :, 0, :],
                op0=OP.mult, op1=OP.add)
            STT(out=ab[:, :, 1, :], in0=x2, scalar=K0, in1=yb[:, :, 1, :],
                op0=OP.mult, op1=OP.add)
            # lam' = max(arg,0) - btil
            STT(out=lam_new[:], in0=arg[:], scalar=0.0, in1=self.btil[:],
                op0=OP.max, op1=OP.subtract)
            # y = (1+b)lam' - b lam  = ts + STT
            ts(out=yl, in0=lam_new[:], scalar1=1.0 + beta, scalar2=None, op0=OP.mult)
            STT(out=yl, in0=lam_prev[:], scalar=-beta, in1=yl,
                op0=OP.mult, op1=OP.add)
        return lams[n_fista % 2]

    # ---------------- finale ----------------
    last_pool = None

    def finale(self, lam_final, out_dram):
        nc = self.nc
        TT = nc.vector.tensor_tensor
        ts = nc.vector.tensor_scalar
        # y tile <- lam_final so forward reads the final lambda
        ts(out=self.yh[:, 0:NL * FH], in0=lam_final[:], scalar1=1.0, scalar2=None, op0=OP.mult)
        self.forward(self.yh, self.SS, self.Qadd, self.X, slack=False)
        # u = K0 * X[0:2] / rs ; 1/rs = ns * rs
        TT(out=self.sc1[:], in0=self.ns[:], in1=self.rs[:], op=OP.mult)
        ts(out=self.sc1[:], in0=self.sc1[:], scalar1=K0, scalar2=None, op0=OP.mult)
        self.last_dve = TT(out=self.opack[:].rearrange("p (f c) -> p c f", c=2),
                           in0=self.X[:, 0:2 * FH].rearrange("p (c f) -> p c f", c=2),
                           in1=self._bc(self.sc1[:], 2), op=OP.mult)
        gsl_rows = out_dram.ap().rearrange("(p f) c -> p f c", p=P)[
            :, self.gi * FH:(self.gi + 1) * FH, :]
        self.out_dma = nc.sync.dma_start(
            out=gsl_rows.rearrange("p f c -> p (f c)"), in_=self.opack[:])


def build_nc(n_power=N_POWER, n_fista=N_FISTA):
    nc = bass.Bass("TRN2")
    din = nc.dram_tensor("packed", [BPC, NFEAT], f32, kind="ExternalInput")
    dout = nc.dram_tensor("u_safe", [BPC, 2], f32, kind="ExternalOutput")

    with tile.TileContext(nc) as tc:
        with ExitStack() as ctx:
            natpool = ctx.enter_context(tc.tile_pool(name="nat", bufs=1))
            natt = natpool.tile([P, F * NFEAT], f32, name="nat", tag="nat")
            in_dma = nc.gpsimd.dma_start(
                out=natt[:], in_=din.ap().rearrange("(p f) a -> p (f a)", p=P))
            gs = [EmitG(ctx, tc, natt, gi) for gi in range(NG)]
            terms = [in_dma]
            for em in gs:
                em.precompute()
                em.power_phase(n_power)
                em.fista_setup()
                lam_final = em.fista(n_fista)
                em.finale(lam_final, dout)
                terms.append(em.last_act)
                if em.last_pool is not None:
                    terms.append(em.last_pool)
                terms.append(em.last_dve)
                terms.append(em.out_dma)
            # exit fence: chain SP NOPs so the tile-exit drain keeps <=1
            # sync wait per instruction (walrus limit)
            for ti in terms:
                nop = nc.sync.nop()
                tile.add_dep_helper(nop.ins, ti.ins, sync=True, reason="exit fence")
    return nc


_NC_CACHE = {}


def _get_nc(n_power=N_POWER, n_fista=N_FISTA):
    key = (n_power, n_fista)
    if key not in _NC_CACHE:
        _NC_CACHE[key] = build_nc(n_power, n_fista)
    return _NC_CACHE[key]


def pack_inputs(inputs, lo, hi):
    n = hi - lo
    cols = [np.asarray(inputs[name], np.float32)[lo:hi].reshape(n, -1)
            for name, _ in RAW_SPECS]
    return np.ascontiguousarray(np.concatenate(cols, axis=1))


def kernel(**inputs):
    nc = _get_nc()
    in_maps = [{"packed": pack_inputs(inputs, c * BPC, (c + 1) * BPC)}
               for c in range(NCORES)]
    res = run_bass_kernel_spmd(nc, in_maps, list(range(NCORES)))
    return np.concatenate([res.results[c]["u_safe"] for c in range(NCORES)],
                          axis=0)


if __name__ == "__main__":
    rng = np.random.default_rng(0)
    demo = {
        "u_nominal": rng.standard_normal((B_FULL, 2)).astype(np.float32),
        "v_current": rng.uniform(0, 1, (B_FULL, 1)).astype(np.float32),
        "p_obs": (2 * rng.standard_normal((B_FULL, MAX_OBS, 2))).astype(np.float32),
        "obs_mask": np.ones((B_FULL, MAX_OBS), np.float32),
        "p_agents": (2 * rng.standard_normal((B_FULL, MAX_NEI, 2))).astype(np.float32),
        "v_agents_local": rng.standard_normal((B_FULL, MAX_NEI, 2)).astype(np.float32),
        "agents_mask": np.ones((B_FULL, MAX_NEI), np.float32),
        "p_c_agent": (2 * rng.standard_normal((B_FULL, 1, 2))).astype(np.float32),
        "v_c_agent": rng.standard_normal((B_FULL, 1, 2)).astype(np.float32),
        "closest_mask": np.ones((B_FULL, 1), np.float32),
    }
    out = kernel(**demo)
    print(out.shape, out.dtype, np.abs(out).max())


# revision 16
# speedup vs baseline: 1.2403x; 1.2403x over previous
"""Trainium2 Bass kernel for the DifferentiableCBFLayer batched dual-FISTA QP.

v2 strategy (pure data parallel, 8 cores x 4096 samples):
  Each core's 4096 samples live as [128 partitions, 32 free] f32 planes.
  The free dim is split into two independent 16-column groups; each group
  runs the full pipeline on its own tiles so the Tile scheduler can
  phase-offset them across engines (DVE + GPSIMD/Pool working together,
  no per-iteration ACT involvement).

  Per-group FISTA iteration (22 active dual rows -- the 3 slack rows are
  provably ~0 (<=2.3e-3 over the whole trajectory) and are dropped from
  the loop; they are kept in the power phase where they shape L):
    fwd:  prod = W o bc(y[0:18]); S01 via log-tree of TT adds (no strided
          tensor_reduce -- measured 1.9 ns/elem vs 1.04 for contiguous TT);
          R2/R3 trees; bx/R01; X = SS o R + Qadd
    bwd:  Bprod = W o bc(X01); pair add; group shifts
    arg:  arg[0:18] = T + y; box rows via 2 STTs (+-K0*X01 + y)
    upd:  lam' = max(arg,0) - btil (STT);
          y' = (1+b)lam' - b lam  (ts mult + STT)
  Heavy TT ops (F1/B1 products) run on the Pool engine; everything else
  on DVE. The power phase normalizes by ||T||^2 (reciprocal, no ACT sqrt)
  which only changes the iterate's scale, not its direction.
"""
import os
from contextlib import ExitStack

import numpy as np

import concourse.bass as bass
import concourse.tile as tile
from concourse import mybir
from concourse.bass_utils import run_bass_kernel_spmd

f32 = mybir.dt.float32
AX = mybir.AxisListType
OP = mybir.AluOpType
AF = mybir.ActivationFunctionType

P = 128
F = 32                 # samples per partition (full width)
NG = 2                 # groups
FH = F // NG           # samples per partition per group
BPC = P * F            # samples per core
NCORES = 8
B_FULL = BPC * NCORES  # 32768

N_POWER = 30
N_FISTA = 300
USE_POOL = True

MAX_OBS = 10
MAX_NEI = 7
BIG = 1000.0
PINV = np.array([0.5, 0.5, 1.0 / 200.0, 1.0 / 200.0, 1.0 / 200.0], np.float64)
K0 = float(np.float32(np.sqrt(PINV[0])))      # sqrt(1/2)
PINV2 = float(np.float32(PINV[2]))
SQ2 = float(np.float32(np.sqrt(2.0)))

RAW_SPECS = [
    ("u_nominal", (BPC, 2)),
    ("v_current", (BPC, 1)),
    ("p_obs", (BPC, MAX_OBS, 2)),
    ("obs_mask", (BPC, MAX_OBS)),
    ("p_agents", (BPC, MAX_NEI, 2)),
    ("v_agents_local", (BPC, MAX_NEI, 2)),
    ("agents_mask", (BPC, MAX_NEI)),
    ("p_c_agent", (BPC, 1, 2)),
    ("v_c_agent", (BPC, 1, 2)),
    ("closest_mask", (BPC, 1)),
]
NFEAT = 73
F_U, F_V, F_OBS, F_OM, F_AG, F_VA, F_AM, F_PC, F_VC, F_CM = 0, 2, 3, 23, 33, 47, 61, 68, 70, 72

# FISTA-loop row layout (22 rows): 0-9 obs, 10-16 nei, 17 cn, 18-21 box
# (box order: -a, +a, -w, +w as in the reference).
NL = 22
# power-phase layout keeps all 25 rows: 0-17 geom, 18-20 slack, 21-24 box.


def _betas(n):
    one, half, four = np.float32(1.0), np.float32(0.5), np.float32(4.0)
    tk = np.float32(1.0)
    out = []
    for _ in range(n):
        tk1 = half * (one + np.sqrt(one + four * tk * tk, dtype=np.float32))
        beta = np.float32((tk - one) / tk1)
        out.append(float(beta))
        tk = tk1
    return out


class EmitG:
    """One sample-group (FH free columns) with its own tiles."""

    def __init__(self, ctx, tc, natt, gi):
        self.tc = tc
        self.nc = tc.nc
        self.gi = gi
        self.natt = natt
        pool = ctx.enter_context(tc.tile_pool(name=f"st{gi}", bufs=1))
        T = lambda n, tag: pool.tile([P, n * FH], f32, name=f"{tag}{gi}", tag=f"{tag}{gi}")
        self.W = T(36, "W")          # Abar cols 0,1, (row, col) pair-major
        self.b = T(25, "b")          # unscaled h (power layout)
        self.btil = T(NL, "btil")    # sqrt(s) * b, loop layout
        self.lamA = T(NL, "lamA")
        self.lamB = T(NL, "lamB")
        self.yh = T(25, "yh")        # loop y (22 used); power z (25)
        self.Tbuf = T(25, "Tbuf")    # backward output
        self.arg = T(NL, "arg")
        self.pscr = T(25, "pscr")    # precompute/power scratch
        self.tsc = T(18, "tsc")      # tree scratch
        # Separate fixed product buffers for F1/B1. Pool-sited ops alternate
        # engines by iteration parity, so every buffer hazard is
        # cross-engine and each instruction needs <=1 sync wait (walrus
        # limit; the Pool engine is out-of-order, so same-engine Pool deps
        # would otherwise synthesize a second wait).
        self.prodF = T(36, "prodF")
        self.prodB = T(36, "prodB")
        self.X = T(5, "X")
        self.R = T(5, "R")
        self.SS = T(5, "SS")
        self.SSp = T(5, "SSp")
        self.Qadd = T(5, "Qadd")
        self.cs2 = T(2, "cs2")
        self.bx = T(2, "bx")
        self.u2 = T(2, "u2")
        self.sc1 = T(1, "sc1")
        self.ns = T(1, "ns")
        self.den = T(1, "den")
        self.rn = T(1, "rn")
        self.rs = T(1, "rs")
        self.sstar = T(1, "sstar")
        self.v2 = T(1, "v2")
        self.opack = T(2, "opack")
        self.fence_pool = ctx.enter_context(tc.tile_pool(name=f"fence{gi}", bufs=2))

    # ---------------- helpers ----------------
    def _pl(self, t, i, n=1):
        return t[:, i * FH:(i + n) * FH]

    def _pv(self, t, i, n):
        """[P, n, FH] view of planes [i, i+n)."""
        return t[:, i * FH:(i + n) * FH].rearrange("p (m f) -> p m f", m=n)

    def _bc(self, plane_ap, n):
        return plane_ap.unsqueeze(1).broadcast_to([P, n, FH])

    def _act_fence(self, act_insts):
        ft = self.fence_pool.tile([P, 1], f32, name=f"fence{self.gi}", tag=f"fence{self.gi}")
        ins = self.nc.vector.memset(ft[:], 0.0)
        for ai in act_insts:
            tile.add_dep_helper(ins.ins, ai.ins, sync=True, reason="act fence")
        return ins

    # ---------------- fwd/bwd machinery ----------------
    def tree10(self, z, zoff, out_pl):
        """out = sum of 10 z planes [zoff, zoff+10) via 4 TT adds (DVE)."""
        TT = self.nc.vector.tensor_tensor
        t = self.tsc
        TT(out=self._pv(t, 0, 5), in0=self._pv(z, zoff, 5),
           in1=self._pv(z, zoff + 5, 5), op=OP.add)
        TT(out=self._pv(t, 0, 2), in0=self._pv(t, 0, 2),
           in1=self._pv(t, 2, 2), op=OP.add)
        TT(out=self._pl(t, 0), in0=self._pl(t, 0), in1=self._pl(t, 1), op=OP.add)
        TT(out=out_pl, in0=self._pl(t, 0), in1=self._pl(t, 4), op=OP.add)

    def tree7(self, z, zoff, out_pl):
        """out = sum of 7 z planes [zoff, zoff+7) via 4 TT adds (DVE).
        Uses tsc planes 5-8 (tree10's result planes 0,4 stay live)."""
        TT = self.nc.vector.tensor_tensor
        t = self.tsc
        TT(out=self._pv(t, 5, 3), in0=self._pv(z, zoff, 3),
           in1=self._pv(z, zoff + 3, 3), op=OP.add)
        TT(out=self._pl(t, 5), in0=self._pl(t, 5), in1=self._pl(t, 6), op=OP.add)
        TT(out=self._pl(t, 6), in0=self._pl(t, 7), in1=self._pl(z, zoff + 6), op=OP.add)
        TT(out=out_pl, in0=self._pl(t, 5), in1=self._pl(t, 6), op=OP.add)

    def forward(self, z, SSbuf, Qadd, X, slack, peng=None):
        """X = SS o R(z) (+Qadd). slack=True: 25-row power layout."""
        nc = self.nc
        TT = nc.vector.tensor_tensor
        PE = (peng or nc.vector).tensor_tensor
        prod = self.prodF
        W4 = self.W[:].rearrange("p (r c f) -> p r c f", r=18, c=2)
        P4 = prod[:].rearrange("p (r c f) -> p r c f", r=18, c=2)
        # products for the 18 geometric rows (optionally on Pool)
        PE(out=P4, in0=W4,
           in1=self._pv(z, 0, 18).unsqueeze(2).broadcast_to([P, 18, 2, FH]),
           op=OP.mult)
        # S01 = column sums over 18 rows via tree on (r) axis, c pairs together.
        # prod pair-major planes: row r occupies planes 2r, 2r+1.
        pv = lambda i, n: prod[:, i * FH:(i + n) * FH].rearrange("p (m f) -> p m f", m=n)
        TT(out=self._pv(self.tsc, 0, 18), in0=pv(0, 18),
           in1=pv(18, 18), op=OP.add)                      # 9 rows x 2 planes
        TT(out=self._pv(self.tsc, 0, 8), in0=self._pv(self.tsc, 0, 8),
           in1=self._pv(self.tsc, 8, 8), op=OP.add)        # 4 rows
        TT(out=self._pv(self.tsc, 0, 4), in0=self._pv(self.tsc, 0, 4),
           in1=self._pv(self.tsc, 4, 4), op=OP.add)        # 2 rows
        TT(out=self._pv(self.tsc, 0, 2), in0=self._pv(self.tsc, 0, 2),
           in1=self._pv(self.tsc, 2, 2), op=OP.add)        # 1 row
        TT(out=self.cs2[:].rearrange("p (c f) -> p c f", c=2),
           in0=self._pv(self.tsc, 0, 2), in1=self._pv(self.tsc, 16, 2), op=OP.add)
        # box diffs: loop rows 18-21 (-a,+a,-w,+w) / power rows 21-24
        bo = 18 if not slack else 21
        zbox = z[:, bo * FH:(bo + 4) * FH].rearrange("p (c g f) -> p c g f", c=2, g=2)
        TT(out=self.bx[:].rearrange("p (c f) -> p c f", c=2),
           in0=zbox[:, :, 1, :], in1=zbox[:, :, 0, :], op=OP.subtract)
        nc.vector.scalar_tensor_tensor(
            out=self._pl(self.R, 0, 2), in0=self.bx[:], scalar=K0, in1=self.cs2[:],
            op0=OP.mult, op1=OP.add)
        # R2 = sum(z[0:10]) (+z18 if slack); R3 = sum(z[10:17]) (+z19)
        self.tree10(z, 0, self._pl(self.R, 2))
        self.tree7(z, 10, self._pl(self.R, 3))
        if slack:
            TT(out=self._pl(self.R, 2), in0=self._pl(self.R, 2), in1=self._pl(z, 18), op=OP.add)
            TT(out=self._pl(self.R, 3), in0=self._pl(self.R, 3), in1=self._pl(z, 19), op=OP.add)
            TT(out=self._pl(self.R, 4), in0=self._pl(z, 17), in1=self._pl(z, 20), op=OP.add)
            nR = 5
        else:
            nR = 4
        # X = SS * R (+ Qadd); loop: X4 = SS4 * z17 handled separately
        TT(out=self._pl(X, 0, nR), in0=self._pl(SSbuf, 0, nR), in1=self._pl(self.R, 0, nR), op=OP.mult)
        if not slack:
            TT(out=self._pl(X, 4), in0=self._pl(SSbuf, 4), in1=self._pl(z, 17), op=OP.mult)
        if Qadd is not None:
            TT(out=X[:], in0=X[:], in1=Qadd[:], op=OP.add)

    def backward_geom(self, X, peng=None):
        """Tbuf[0:18] = W o X01 pair-sum minus group shifts."""
        nc = self.nc
        TT = nc.vector.tensor_tensor
        PE = (peng or nc.vector).tensor_tensor
        Tb = self.Tbuf
        prod = self.prodB
        W4 = self.W[:].rearrange("p (r c f) -> p r c f", r=18, c=2)
        P4 = prod[:].rearrange("p (r c f) -> p r c f", r=18, c=2)
        x2 = X[:, 0:2 * FH].rearrange("p (c f) -> p c f", c=2)
        PE(out=P4, in0=W4,
           in1=x2.unsqueeze(1).broadcast_to([P, 18, 2, FH]), op=OP.mult)
        TT(out=self._pv(Tb, 0, 18),
           in0=P4[:, :, 0, :], in1=P4[:, :, 1, :], op=OP.add)
        TT(out=self._pv(Tb, 0, 10), in0=self._pv(Tb, 0, 10),
           in1=self._bc(self._pl(X, 2), 10), op=OP.subtract)
        TT(out=self._pv(Tb, 10, 7), in0=self._pv(Tb, 10, 7),
           in1=self._bc(self._pl(X, 3), 7), op=OP.subtract)
        TT(out=self._pl(Tb, 17), in0=self._pl(Tb, 17), in1=self._pl(X, 4), op=OP.subtract)

    def backward_tail_power(self, X):
        """Power-phase slack+box rows of Tbuf (25-row layout) on DVE."""
        nc = self.nc
        ts = nc.vector.tensor_scalar
        ts(out=self._pl(self.Tbuf, 18, 3), in0=self._pl(X, 2, 3), scalar1=-1.0,
           scalar2=None, op0=OP.mult)
        tb = self.Tbuf[:, 21 * FH:25 * FH].rearrange("p (c g f) -> p c g f", c=2, g=2)
        x2 = X[:, 0:2 * FH].rearrange("p (c f) -> p c f", c=2)
        ts(out=tb[:, :, 0, :], in0=x2, scalar1=-K0, scalar2=None, op0=OP.mult)
        ts(out=tb[:, :, 1, :], in0=x2, scalar1=K0, scalar2=None, op0=OP.mult)

    # ---------------- precompute (per group) ----------------
    def precompute(self):
        nc = self.nc
        gsl = slice(self.gi * FH, (self.gi + 1) * FH)
        Wv = self.W[:].rearrange("p (m f) -> p m f", m=36)
        bv = self.b[:].rearrange("p (m f) -> p m f", m=25)
        STT = nc.vector.scalar_tensor_tensor
        TT = nc.vector.tensor_tensor
        TS = nc.vector.tensor_scalar

        pk = self.natt[:].rearrange("p (f a) -> p a f", a=NFEAT)[:, :, gsl]
        self.pk = pk
        obs = pk[:, F_OBS:F_OBS + 20, :].rearrange("p (o c) f -> p o c f", c=2)
        lx, ly = obs[:, :, 0, :], obs[:, :, 1, :]
        om = pk[:, F_OM:F_OM + MAX_OBS, :]
        ag = pk[:, F_AG:F_AG + 14, :].rearrange("p (o c) f -> p o c f", c=2)
        ax, ay = ag[:, :, 0, :], ag[:, :, 1, :]
        va = pk[:, F_VA:F_VA + 14, :].rearrange("p (o c) f -> p o c f", c=2)
        vjx, vjy = va[:, :, 0, :], va[:, :, 1, :]
        am = pk[:, F_AM:F_AM + MAX_NEI, :]
        cx, cy = pk[:, F_PC, :], pk[:, F_PC + 1, :]
        cvx, cvy = pk[:, F_VC, :], pk[:, F_VC + 1, :]
        cm = pk[:, F_CM, :]
        v = pk[:, F_V, :]

        STT(out=self.v2[:], in0=v, scalar=2.0, in1=v, op0=OP.mult, op1=OP.mult)

        sA = self._pv(self.pscr, 0, 10)
        sB = self._pv(self.pscr, 10, 10)

        # ---- obs rows (0-9; W pairs 0..19) ----
        W0 = Wv[:, 0:20].rearrange("p (o c) f -> p o c f", c=2)[:, :, 0, :]
        W1 = Wv[:, 0:20].rearrange("p (o c) f -> p o c f", c=2)[:, :, 1, :]
        STT(out=W0, in0=lx, scalar=2.0 * K0, in1=om, op0=OP.mult, op1=OP.mult)
        STT(out=sA, in0=ly, scalar=2.0 * K0, in1=self._bc(v, 10), op0=OP.mult, op1=OP.mult)
        TT(out=W1, in0=sA, in1=om, op=OP.mult)
        STT(out=sA, in0=lx, scalar=-6.0, in1=self._bc(v, 10), op0=OP.mult, op1=OP.mult)
        STT(out=sB, in0=lx, scalar=2.0, in1=lx, op0=OP.mult, op1=OP.mult)
        TT(out=sA, in0=sA, in1=sB, op=OP.add)
        STT(out=sB, in0=ly, scalar=2.0, in1=ly, op0=OP.mult, op1=OP.mult)
        TT(out=sA, in0=sA, in1=sB, op=OP.add)
        TT(out=sA, in0=sA, in1=self._bc(self.v2[:], 10), op=OP.add)
        TS(out=sA, in0=sA, scalar1=-0.5, scalar2=None, op0=OP.add)
        STT(out=sA, in0=sA, scalar=-BIG, in1=om, op0=OP.add, op1=OP.mult)
        TS(out=bv[:, 0:10], in0=sA, scalar1=BIG, scalar2=None, op0=OP.add)

        # ---- nei rows (10-16; W pairs 20..33) ----
        sA7 = self._pv(self.pscr, 0, 7)
        sB7 = self._pv(self.pscr, 7, 7)
        W0 = Wv[:, 20:34].rearrange("p (o c) f -> p o c f", c=2)[:, :, 0, :]
        W1 = Wv[:, 20:34].rearrange("p (o c) f -> p o c f", c=2)[:, :, 1, :]
        STT(out=W0, in0=ax, scalar=2.0 * K0, in1=am, op0=OP.mult, op1=OP.mult)
        STT(out=sA7, in0=ay, scalar=2.0 * K0, in1=self._bc(v, 7), op0=OP.mult, op1=OP.mult)
        STT(out=sB7, in0=ay, scalar=-2.0 * K0, in1=vjx, op0=OP.mult, op1=OP.mult)
        TT(out=sA7, in0=sA7, in1=sB7, op=OP.add)
        STT(out=sB7, in0=ax, scalar=2.0 * K0, in1=vjy, op0=OP.mult, op1=OP.mult)
        TT(out=sA7, in0=sA7, in1=sB7, op=OP.add)
        TT(out=W1, in0=sA7, in1=am, op=OP.mult)
        STT(out=sA7, in0=vjx, scalar=-4.0, in1=self._bc(v, 7), op0=OP.mult, op1=OP.mult)
        STT(out=sB7, in0=vjx, scalar=2.0, in1=vjx, op0=OP.mult, op1=OP.mult)
        TT(out=sA7, in0=sA7, in1=sB7, op=OP.add)
        STT(out=sB7, in0=vjy, scalar=2.0, in1=vjy, op0=OP.mult, op1=OP.mult)
        TT(out=sA7, in0=sA7, in1=sB7, op=OP.add)
        STT(out=sB7, in0=ax, scalar=-6.0, in1=self._bc(v, 7), op0=OP.mult, op1=OP.mult)
        TT(out=sA7, in0=sA7, in1=sB7, op=OP.add)
        STT(out=sB7, in0=ax, scalar=6.0, in1=vjx, op0=OP.mult, op1=OP.mult)
        TT(out=sA7, in0=sA7, in1=sB7, op=OP.add)
        STT(out=sB7, in0=ay, scalar=6.0, in1=vjy, op0=OP.mult, op1=OP.mult)
        TT(out=sA7, in0=sA7, in1=sB7, op=OP.add)
        STT(out=sB7, in0=ax, scalar=2.0, in1=ax, op0=OP.mult, op1=OP.mult)
        TT(out=sA7, in0=sA7, in1=sB7, op=OP.add)
        STT(out=sB7, in0=ay, scalar=2.0, in1=ay, op0=OP.mult, op1=OP.mult)
        TT(out=sA7, in0=sA7, in1=sB7, op=OP.add)
        TT(out=sA7, in0=sA7, in1=self._bc(self.v2[:], 7), op=OP.add)
        TS(out=sA7, in0=sA7, scalar1=-1.28, scalar2=None, op0=OP.add)
        STT(out=sA7, in0=sA7, scalar=-BIG, in1=am, op0=OP.add, op1=OP.mult)
        TS(out=bv[:, 10:17], in0=sA7, scalar1=BIG, scalar2=None, op0=OP.add)

        # ---- cn row (17; W pair 34,35) ----
        s1 = self._pl(self.pscr, 0)
        s2 = self._pl(self.pscr, 1)
        STT(out=Wv[:, 34], in0=cx, scalar=-2.0 * K0, in1=cm, op0=OP.mult, op1=OP.mult)
        STT(out=s1, in0=cy, scalar=-2.0 * K0, in1=v, op0=OP.mult, op1=OP.mult)
        STT(out=s2, in0=cy, scalar=2.0 * K0, in1=cvx, op0=OP.mult, op1=OP.mult)
        TT(out=s1, in0=s1, in1=s2, op=OP.add)
        STT(out=s2, in0=cx, scalar=-2.0 * K0, in1=cvy, op0=OP.mult, op1=OP.mult)
        TT(out=s1, in0=s1, in1=s2, op=OP.add)
        TT(out=Wv[:, 35], in0=s1, in1=cm, op=OP.mult)
        STT(out=s1, in0=cvx, scalar=4.0, in1=v, op0=OP.mult, op1=OP.mult)
        STT(out=s2, in0=cvx, scalar=-2.0, in1=cvx, op0=OP.mult, op1=OP.mult)
        TT(out=s1, in0=s1, in1=s2, op=OP.add)
        STT(out=s2, in0=cvy, scalar=-2.0, in1=cvy, op0=OP.mult, op1=OP.mult)
        TT(out=s1, in0=s1, in1=s2, op=OP.add)
        STT(out=s2, in0=cx, scalar=6.0, in1=v, op0=OP.mult, op1=OP.mult)
        TT(out=s1, in0=s1, in1=s2, op=OP.add)
        STT(out=s2, in0=cx, scalar=-6.0, in1=cvx, op0=OP.mult, op1=OP.mult)
        TT(out=s1, in0=s1, in1=s2, op=OP.add)
        STT(out=s2, in0=cy, scalar=-6.0, in1=cvy, op0=OP.mult, op1=OP.mult)
        TT(out=s1, in0=s1, in1=s2, op=OP.add)
        STT(out=s2, in0=cx, scalar=-2.0, in1=cx, op0=OP.mult, op1=OP.mult)
        TT(out=s1, in0=s1, in1=s2, op=OP.add)
        STT(out=s2, in0=cy, scalar=-2.0, in1=cy, op0=OP.mult, op1=OP.mult)
        TT(out=s1, in0=s1, in1=s2, op=OP.add)
        TT(out=s1, in0=s1, in1=self.v2[:], op=OP.subtract)
        TS(out=s1, in0=s1, scalar1=50.0, scalar2=None, op0=OP.add)
        STT(out=s1, in0=s1, scalar=-BIG, in1=cm, op0=OP.add, op1=OP.mult)
        TS(out=self._pl(self.b, 17), in0=s1, scalar1=BIG, scalar2=None, op0=OP.add)

        # ---- slack/box b rows (power layout), power scale planes ----
        nc.vector.memset(self._pl(self.b, 18, 3), 0.0)
        nc.vector.memset(self._pl(self.b, 21, 2), 2.0)
        nc.vector.memset(self._pl(self.b, 23, 2), 1.0)
        nc.vector.memset(self._pl(self.SSp, 0, 2), 1.0)
        nc.vector.memset(self._pl(self.SSp, 2, 3), -PINV2)

    # ---------------- power phase ----------------
    def power_init(self):
        self.nc.vector.memset(self.yh[:], 1.0)

    def power_iter(self, it):
        nc = self.nc
        TT = nc.vector.tensor_tensor
        sq = self.pscr[:, 0:25 * FH]
        par = (it + self.gi) % 2
        f1eng = nc.gpsimd if (USE_POOL and par == 0) else None
        b1eng = nc.gpsimd if (USE_POOL and par == 1) else None
        self.forward(self.yh, self.SSp, None, self.X, slack=True, peng=f1eng)
        self.backward_geom(self.X, peng=b1eng)
        self.backward_tail_power(self.X)
        # z' = T / ||T||^2  (direction-preserving, no sqrt needed)
        TT(out=sq, in0=self.Tbuf[:], in1=self.Tbuf[:], op=OP.mult)
        nc.vector.tensor_reduce(
            out=self.ns[:], in_=sq.rearrange("p (m f) -> p f m", m=25),
            axis=AX.X, op=OP.add)
        nc.vector.reciprocal(out=self.rn[:], in_=self.ns[:])
        TT(out=self._pv(self.yh, 0, 25), in0=self._pv(self.Tbuf, 0, 25),
           in1=self._bc(self.rn[:], 25), op=OP.mult)

    def power_final(self):
        # Rayleigh: L = (z.Mz)/(z.z); ns := L + 1e-6; rs = rsqrt(ns)
        nc = self.nc
        TT = nc.vector.tensor_tensor
        sq = self.pscr[:, 0:25 * FH]
        self.forward(self.yh, self.SSp, None, self.X, slack=True)
        self.backward_geom(self.X)
        self.backward_tail_power(self.X)
        TT(out=sq, in0=self.yh[:], in1=self.Tbuf[:], op=OP.mult)
        nc.vector.tensor_reduce(
            out=self.ns[:], in_=sq.rearrange("p (m f) -> p f m", m=25),
            axis=AX.X, op=OP.add)
        TT(out=sq, in0=self.yh[:], in1=self.yh[:], op=OP.mult)
        nc.vector.tensor_reduce(
            out=self.den[:], in_=sq.rearrange("p (m f) -> p f m", m=25),
            axis=AX.X, op=OP.add)
        nc.vector.reciprocal(out=self.rn[:], in_=self.den[:])
        TT(out=self.ns[:], in0=self.ns[:], in1=self.rn[:], op=OP.mult)
        nc.vector.tensor_scalar(out=self.ns[:], in0=self.ns[:],
                                scalar1=1e-6, scalar2=None, op0=OP.add)
        self.emit_rsqrt(self.rs[:], self.ns[:], newton=2)
        TT(out=self.sstar[:], in0=self.rs[:], in1=self.rs[:], op=OP.mult)

    def emit_rsqrt(self, dst, src, newton=0):
        nc = self.nc
        nc.vector.reciprocal(out=self.sc1[:], in_=src)
        a = nc.scalar.activation(dst, self.sc1[:], AF.Sqrt)
        self.last_act = a
        self._act_fence([a])
        for _ in range(newton):
            nc.vector.tensor_tensor(out=self.sc1[:], in0=dst, in1=dst, op=OP.mult)
            nc.vector.tensor_tensor(out=self.sc1[:], in0=src, in1=self.sc1[:], op=OP.mult)
            nc.vector.tensor_scalar(out=self.sc1[:], in0=self.sc1[:],
                                    scalar1=-0.5, scalar2=1.5, op0=OP.mult, op1=OP.add)
            nc.vector.tensor_tensor(out=dst, in0=dst, in1=self.sc1[:], op=OP.mult)

    # ---------------- FISTA setup ----------------
    def fista_setup(self):
        """btil (22-row layout), SS, Qadd, lam0=y0=-btil."""
        nc = self.nc
        TT = nc.vector.tensor_tensor
        ts = nc.vector.tensor_scalar
        # btil loop layout: rows 0-17 from b[0:18], rows 18-21 from b[21:25]
        TT(out=self._pv(self.btil, 0, 18), in0=self._pv(self.b, 0, 18),
           in1=self._bc(self.rs[:], 18), op=OP.mult)
        TT(out=self._pv(self.btil, 18, 4), in0=self._pv(self.b, 21, 4),
           in1=self._bc(self.rs[:], 4), op=OP.mult)
        # SS = [-s*, -s*, PINV2*s* x3]
        ts(out=self._pv(self.SS, 0, 2), in0=self._bc(self.sstar[:], 2),
           scalar1=-1.0, scalar2=None, op0=OP.mult)
        ts(out=self._pv(self.SS, 2, 3), in0=self._bc(self.sstar[:], 3),
           scalar1=PINV2, scalar2=None, op0=OP.mult)
        # Qadd = SS*FWD25(btil-as-25rows) + rs*q~ ; build a 25-row view of btil
        # (slack rows zero): reuse Tbuf as scratch z.
        nc.vector.memset(self.Tbuf[:], 0.0)
        ts(out=self._pv(self.Tbuf, 0, 18), in0=self._pv(self.btil, 0, 18),
           scalar1=1.0, scalar2=None, op0=OP.mult)
        ts(out=self._pv(self.Tbuf, 21, 4), in0=self._pv(self.btil, 18, 4),
           scalar1=1.0, scalar2=None, op0=OP.mult)
        self.forward(self.Tbuf, self.SS, None, self.Qadd, slack=True)
        uap = self.pk[:, F_U:F_U + 2, :]
        nc.vector.scalar_tensor_tensor(
            out=self.u2[:].rearrange("p (c f) -> p c f", c=2),
            in0=uap, scalar=SQ2, in1=self._bc(self.rs[:], 2), op0=OP.mult, op1=OP.mult)
        TT(out=self._pl(self.Qadd, 0, 2), in0=self._pl(self.Qadd, 0, 2), in1=self.u2[:],
           op=OP.add)
        # lam = y = -btil (22 rows)
        ts(out=self.lamA[:, 0:NL * FH], in0=self.btil[:], scalar1=-1.0, scalar2=None, op0=OP.mult)
        ts(out=self.yh[:, 0:NL * FH], in0=self.btil[:], scalar1=-1.0, scalar2=None, op0=OP.mult)

    # ---------------- FISTA loop ----------------
    def fista_iter(self, it, betas):
        nc = self.nc
        TT = nc.vector.tensor_tensor
        STT = nc.vector.scalar_tensor_tensor
        ts = nc.vector.tensor_scalar
        lams = [self.lamA, self.lamB]
        yl = self.yh[:, 0:NL * FH]
        if True:
            lam_prev = lams[it % 2]
            lam_new = lams[(it + 1) % 2]
            beta = float(np.float32(betas[it]))
            par = (it + self.gi) % 2
            f1eng = nc.gpsimd if (USE_POOL and par == 0) else None
            b1eng = nc.gpsimd if (USE_POOL and par == 1) else None
            self.forward(self.yh, self.SS, self.Qadd, self.X, slack=False,
                         peng=f1eng)
            self.backward_geom(self.X, peng=b1eng)
            arg = self.arg
            # arg rows 0-17: T + y  (alternates DVE/Pool by parity)
            u1eng = nc.gpsimd if (USE_POOL and par == 1) else nc.vector
            ins = u1eng.tensor_tensor(
                out=arg[:, 0:18 * FH].rearrange("p (m f) -> p m f", m=18),
                in0=self._pv(self.Tbuf, 0, 18),
                in1=self._pv(self.yh, 0, 18), op=OP.add)
            if USE_POOL and par == 1:
                self.last_pool = ins
            # arg box rows: -+K0*X01 + y  (planes 18-21 as [c=a/w, g=-/+])
            x2 = self.X[:, 0:2 * FH].rearrange("p (c f) -> p c f", c=2)
            ab = arg[:, 18 * FH:22 * FH].rearrange("p (c g f) -> p c g f", c=2, g=2)
            yb = self.yh[:, 18 * FH:22 * FH].rearrange("p (c g f) -> p c g f", c=2, g=2)
            STT(out=ab[:, :, 0, :], in0=x2, scalar=-K0, in1=yb[:, :, 0, :],
                op0=OP.mult, op1=OP.add)
            STT(out=ab[:, :, 1, :], in0=x2, scalar=K0, in1=yb[:, :, 1, :],
                op0=OP.mult, op1=OP.add)
            # lam' = max(arg,0) - btil
            STT(out=lam_new[:], in0=arg[:], scalar=0.0, in1=self.btil[:],
                op0=OP.max, op1=OP.subtract)
            # y = (1+b)lam' - b lam  = ts + STT
            ts(out=yl, in0=lam_new[:], scalar1=1.0 + beta, scalar2=None, op0=OP.mult)
            STT(out=yl, in0=lam_prev[:], scalar=-beta, in1=yl,
                op0=OP.mult, op1=OP.add)

    def fista_result(self, n_fista):
        return [self.lamA, self.lamB][n_fista % 2]

    # ---------------- finale ----------------
    last_pool = None

    def finale(self, lam_final, out_dram):
        nc = self.nc
        TT = nc.vector.tensor_tensor
        ts = nc.vector.tensor_scalar
        # y tile <- lam_final so forward reads the final lambda
        ts(out=self.yh[:, 0:NL * FH], in0=lam_final[:], scalar1=1.0, scalar2=None, op0=OP.mult)
        self.forward(self.yh, self.SS, self.Qadd, self.X, slack=False)
        # u = K0 * X[0:2] / rs ; 1/rs = ns * rs
        TT(out=self.sc1[:], in0=self.ns[:], in1=self.rs[:], op=OP.mult)
        ts(out=self.sc1[:], in0=self.sc1[:], scalar1=K0, scalar2=None, op0=OP.mult)
        self.last_dve = TT(out=self.opack[:].rearrange("p (f c) -> p c f", c=2),
                           in0=self.X[:, 0:2 * FH].rearrange("p (c f) -> p c f", c=2),
                           in1=self._bc(self.sc1[:], 2), op=OP.mult)
        gsl_rows = out_dram.ap().rearrange("(p f) c -> p f c", p=P)[
            :, self.gi * FH:(self.gi + 1) * FH, :]
        self.out_dma = nc.sync.dma_start(
            out=gsl_rows.rearrange("p f c -> p (f c)"), in_=self.opack[:])


def build_nc(n_power=N_POWER, n_fista=N_FISTA):
    nc = bass.Bass("TRN2")
    din = nc.dram_tensor("packed", [BPC, NFEAT], f32, kind="ExternalInput")
    dout = nc.dram_tensor("u_safe", [BPC, 2], f32, kind="ExternalOutput")

    with tile.TileContext(nc) as tc:
        with ExitStack() as ctx:
            natpool = ctx.enter_context(tc.tile_pool(name="nat", bufs=1))
            natt = natpool.tile([P, F * NFEAT], f32, name="nat", tag="nat")
            in_dma = nc.gpsimd.dma_start(
                out=natt[:], in_=din.ap().rearrange("(p f) a -> p (f a)", p=P))
            gs = [EmitG(ctx, tc, natt, gi) for gi in range(NG)]
            terms = [in_dma]
            betas = _betas(n_fista)
            for em in gs:
                em.precompute()
            for em in gs:
                em.power_init()
            for it in range(n_power):
                for em in gs:
                    em.power_iter(it)
            for em in gs:
                em.power_final()
            for em in gs:
                em.fista_setup()
            for it in range(n_fista):
                for em in gs:
                    em.fista_iter(it, betas)
            for em in gs:
                em.finale(em.fista_result(n_fista), dout)
                terms.append(em.last_act)
                if em.last_pool is not None:
                    terms.append(em.last_pool)
                terms.append(em.last_dve)
                terms.append(em.out_dma)
            # exit fence: chain SP NOPs so the tile-exit drain keeps <=1
            # sync wait per instruction (walrus limit)
            for ti in terms:
                nop = nc.sync.nop()
                tile.add_dep_helper(nop.ins, ti.ins, sync=True, reason="exit fence")
    return nc


_NC_CACHE = {}


def _get_nc(n_power=N_POWER, n_fista=N_FISTA):
    key = (n_power, n_fista)
    if key not in _NC_CACHE:
        _NC_CACHE[key] = build_nc(n_power, n_fista)
    return _NC_CACHE[key]


def pack_inputs(inputs, lo, hi):
    n = hi - lo
    cols = [np.asarray(inputs[name], np.float32)[lo:hi].reshape(n, -1)
            for name, _ in RAW_SPECS]
    return np.ascontiguousarray(np.concatenate(cols, axis=1))


def kernel(**inputs):
    nc = _get_nc()
    in_maps = [{"packed": pack_inputs(inputs, c * BPC, (c + 1) * BPC)}
               for c in range(NCORES)]
    res = run_bass_kernel_spmd(nc, in_maps, list(range(NCORES)))
    return np.concatenate([res.results[c]["u_safe"] for c in range(NCORES)],
                          axis=0)


if __name__ == "__main__":
    rng = np.random.default_rng(0)
    demo = {
        "u_nominal": rng.standard_normal((B_FULL, 2)).astype(np.float32),
        "v_current": rng.uniform(0, 1, (B_FULL, 1)).astype(np.float32),
        "p_obs": (2 * rng.standard_normal((B_FULL, MAX_OBS, 2))).astype(np.float32),
        "obs_mask": np.ones((B_FULL, MAX_OBS), np.float32),
        "p_agents": (2 * rng.standard_normal((B_FULL, MAX_NEI, 2))).astype(np.float32),
        "v_agents_local": rng.standard_normal((B_FULL, MAX_NEI, 2)).astype(np.float32),
        "agents_mask": np.ones((B_FULL, MAX_NEI), np.float32),
        "p_c_agent": (2 * rng.standard_normal((B_FULL, 1, 2))).astype(np.float32),
        "v_c_agent": rng.standard_normal((B_FULL, 1, 2)).astype(np.float32),
        "closest_mask": np.ones((B_FULL, 1), np.float32),
    }
    out = kernel(**demo)
    print(out.shape, out.dtype, np.abs(out).max())
